# revision 1
# baseline (speedup 1.0000x reference)
# Trainium2 Bass kernel for nn_Connection_geognn_17076789969601.
#
# Math (per sample row of input_ [N, 128], x = row[:64], v = row[64:]):
#   h  = tanh(W1 @ x + b1)                  # [128]
#   Wm = tanh(W2 @ h + b2).reshape(64, 4)   # [64, 4]
#   u  = v @ Wm                             # [4]
#   H  = sum(u^2)
#   g  = dH/d(row);  output = [g[:64], -g[64:]]
#
# Backward (per sample):
#   dWm   = 2 v u^T ;  dv = 2 Wm u  (output_v = -dv)
#   dA2   = (2 u ⊗ v) * (1 - T2^2)   with T2 = tanh(A2) (W2-rows permuted so
#           Wm column j = rows [64j, 64j+64) of the permuted T2)
#   dh    = dA2 @ W2r ; dA1 = dh * (1 - h^2) ; dx = dA1 @ W1  (output_x = dx)
#
# Layout on device: feature-major ("transposed") activations [feat, samples],
# samples on the free axis, 1024 samples per macro tile.  Cross-partition
# reductions/broadcasts (u over 64-blocks, V replicated to 128 partitions) are
# done with small constant mask matmuls on the tensor engine.  Signs/scales are
# folded into host-precomputed constants:
#   Mblk entries = +2.0   -> R = Mblk @ (V*T2) = 2*u_rep            (PSUM)
#   Msum entries = -1.0   -> dV = Msum @ (R*T2) = -2*Wm@u = out_v   (PSUM)
#   Kneg = (T2^2 - 1) * Vrep  (fused DVE op) ; dA2m = R*Kneg = -dA2
#   lhsT for dh = -W2r chunks -> dh exact ; dA1m = (h^2-1)*dh = -dA1
#   lhsT for dx = -W1 -> dx exact.
#
# Sharding: pure data parallel over 8 NeuronCores, batch split 262144 -> 8 x
# 32768, weights replicated.

import sys

sys.path.insert(0, "/opt/trn_rl_repo")

import numpy as np
import ml_dtypes

import concourse.bass as bass
import concourse.bacc as bacc
import concourse.tile as tile
import concourse.mybir as mybir
from concourse.bass_utils import run_bass_kernel_spmd

F32 = mybir.dt.float32
BF16 = mybir.dt.bfloat16
AF = mybir.ActivationFunctionType
ALU = mybir.AluOpType

D = 64
RANK = 4
N_TOTAL = 262144
N_CORES = 8
N_ROWS = N_TOTAL // N_CORES  # 32768 per core
B = 1024                     # samples per macro tile
G = B // 128                 # 128-sample groups per tile


def build_program(n_rows=N_ROWS, b=B):
    g = b // 128
    nt = n_rows // b
    nc = bacc.Bacc()

    inp = nc.declare_dram_parameter("inp", [n_rows, 128], F32, isOutput=False)
    w1t = nc.declare_dram_parameter("w1t", [64, 128], BF16, isOutput=False)
    w2ta = nc.declare_dram_parameter("w2ta", [128, 128], BF16, isOutput=False)
    w2tb = nc.declare_dram_parameter("w2tb", [128, 128], BF16, isOutput=False)
    w2na = nc.declare_dram_parameter("w2na", [128, 128], BF16, isOutput=False)
    w2nb = nc.declare_dram_parameter("w2nb", [128, 128], BF16, isOutput=False)
    w1n = nc.declare_dram_parameter("w1n", [128, 64], BF16, isOutput=False)
    mblk = nc.declare_dram_parameter("mblk", [128, 128], BF16, isOutput=False)
    msum = nc.declare_dram_parameter("msum", [128, 64], BF16, isOutput=False)
    ident = nc.declare_dram_parameter("ident", [128, 128], F32, isOutput=False)
    b1p = nc.declare_dram_parameter("b1", [128, 1], F32, isOutput=False)
    b2ap = nc.declare_dram_parameter("b2a", [128, 1], F32, isOutput=False)
    b2bp = nc.declare_dram_parameter("b2b", [128, 1], F32, isOutput=False)
    outp = nc.declare_dram_parameter("out", [n_rows, 128], F32, isOutput=True)

    with tile.TileContext(nc) as tc:
        with (
            tc.tile_pool(name="const", bufs=1) as cp,
            tc.tile_pool(name="sb", bufs=2) as sb,
            tc.tile_pool(name="psA", bufs=2, space="PSUM") as psA,
            tc.tile_pool(name="psB", bufs=2, space="PSUM") as psB,
        ):
            c_w1t = cp.tile([64, 128], BF16, tag="w1t")
            c_w2ta = cp.tile([128, 128], BF16, tag="w2ta")
            c_w2tb = cp.tile([128, 128], BF16, tag="w2tb")
            c_w2na = cp.tile([128, 128], BF16, tag="w2na")
            c_w2nb = cp.tile([128, 128], BF16, tag="w2nb")
            c_w1n = cp.tile([128, 64], BF16, tag="w1n")
            c_mblk = cp.tile([128, 128], BF16, tag="mblk")
            c_msum = cp.tile([128, 64], BF16, tag="msum")
            c_id = cp.tile([128, 128], F32, tag="ident")
            c_b1 = cp.tile([128, 1], F32, tag="b1")
            c_b2a = cp.tile([128, 1], F32, tag="b2a")
            c_b2b = cp.tile([128, 1], F32, tag="b2b")
            for t_, p_ in (
                (c_w1t, w1t), (c_w2ta, w2ta), (c_w2tb, w2tb), (c_w2na, w2na),
                (c_w2nb, w2nb), (c_w1n, w1n), (c_mblk, mblk), (c_msum, msum),
                (c_id, ident), (c_b1, b1p), (c_b2a, b2ap), (c_b2b, b2bp),
            ):
                nc.sync.dma_start(t_[:], p_[:])

            for t in range(nt):
                # ---- load + transpose input tile ----
                tin = sb.tile([128, g, 128], F32, tag="IN")
                nc.sync.dma_start(
                    tin[:, :, :],
                    inp[bass.ts(t, b), :].rearrange("(g p) f -> p g f", p=128),
                )
                tp = psA.tile([128, b], F32, tag="psA")
                for k in range(g):
                    nc.tensor.transpose(tp[:, bass.ts(k, 128)], tin[:, k, :], c_id[:])
                tint = sb.tile([128, b], BF16, tag="INT")  # [x^T; v^T] bf16
                nc.vector.tensor_copy(tint[:], tp[:])
                vrep = sb.tile([128, b], BF16, tag="VT")   # [v^T; v^T]
                nc.sync.dma_start(vrep[0:64, :], tint[64:128, :])
                nc.sync.dma_start(vrep[64:128, :], tint[64:128, :])

                # ---- forward layer 1 ----
                a1 = psB.tile([128, b], F32, tag="psB")
                for h in range(b // 512):
                    nc.tensor.matmul(a1[:, bass.ts(h, 512)], c_w1t[:],
                                     tint[0:64, bass.ts(h, 512)],
                                     start=True, stop=True)
                h1 = sb.tile([128, b], BF16, tag="H1")
                nc.scalar.activation(h1[:], a1[:], AF.Tanh, bias=c_b1[:, 0:1])

                # ---- forward layer 2 (W2 rows permuted; two 128-row halves) ----
                a2a = psA.tile([128, b], F32, tag="psA")
                a2b = psB.tile([128, b], F32, tag="psB")
                for h in range(b // 512):
                    nc.tensor.matmul(a2a[:, bass.ts(h, 512)], c_w2ta[:],
                                     h1[:, bass.ts(h, 512)], start=True, stop=True)
                    nc.tensor.matmul(a2b[:, bass.ts(h, 512)], c_w2tb[:],
                                     h1[:, bass.ts(h, 512)], start=True, stop=True)
                t2a = sb.tile([128, b], BF16, tag="T2a")
                t2b = sb.tile([128, b], BF16, tag="T2b")
                nc.scalar.activation(t2a[:], a2a[:], AF.Tanh, bias=c_b2a[:, 0:1])
                nc.scalar.activation(t2b[:], a2b[:], AF.Tanh, bias=c_b2b[:, 0:1])

                # ---- u (block-sum+broadcast via mask matmul): R = 2*u_rep ----
                pa = sb.tile([128, b], BF16, tag="Pa")
                pb = sb.tile([128, b], BF16, tag="Pb")
                nc.vector.tensor_mul(pa[:], vrep[:], t2a[:])
                nc.vector.tensor_mul(pb[:], vrep[:], t2b[:])
                ra = psA.tile([128, b], F32, tag="psA")
                rb = psB.tile([128, b], F32, tag="psB")
                for h in range(b // 512):
                    nc.tensor.matmul(ra[:, bass.ts(h, 512)], c_mblk[:],
                                     pa[:, bass.ts(h, 512)], start=True, stop=True)
                    nc.tensor.matmul(rb[:, bass.ts(h, 512)], c_mblk[:],
                                     pb[:, bass.ts(h, 512)], start=True, stop=True)

                # ---- dv (output v-part, sign folded into msum) ----
                sa = sb.tile([128, b], BF16, tag="Sa")
                sbt = sb.tile([128, b], BF16, tag="Sb")
                nc.vector.tensor_mul(sa[:], ra[:], t2a[:])
                nc.vector.tensor_mul(sbt[:], rb[:], t2b[:])
                dv = psA.tile([64, b], F32, tag="psA")
                for h in range(b // 512):
                    nc.tensor.matmul(dv[:, bass.ts(h, 512)], c_msum[:],
                                     sa[:, bass.ts(h, 512)], start=True, stop=False)
                    nc.tensor.matmul(dv[:, bass.ts(h, 512)], c_msum[:],
                                     sbt[:, bass.ts(h, 512)], start=False, stop=True)

                # ---- dA2 (negated): dA2m = R * (T2^2 - 1) * Vrep ----
                t2sqa = sb.tile([128, b], BF16, tag="T2sqa")
                t2sqb = sb.tile([128, b], BF16, tag="T2sqb")
                nc.scalar.activation(t2sqa[:], t2a[:], AF.Square)
                nc.scalar.activation(t2sqb[:], t2b[:], AF.Square)
                knega = sb.tile([128, b], BF16, tag="Knega")
                knegb = sb.tile([128, b], BF16, tag="Knegb")
                nc.vector.scalar_tensor_tensor(
                    knega[:], t2sqa[:], 1.0, vrep[:], ALU.subtract, ALU.mult)
                nc.vector.scalar_tensor_tensor(
                    knegb[:], t2sqb[:], 1.0, vrep[:], ALU.subtract, ALU.mult)
                da2a = sb.tile([128, b], BF16, tag="dA2a")
                da2b = sb.tile([128, b], BF16, tag="dA2b")
                nc.vector.tensor_mul(da2a[:], ra[:], knega[:])
                nc.vector.tensor_mul(da2b[:], rb[:], knegb[:])

                # ---- backward layer 1 ----
                dh1 = psB.tile([128, b], F32, tag="psB")
                for h in range(b // 512):
                    nc.tensor.matmul(dh1[:, bass.ts(h, 512)], c_w2na[:],
                                     da2a[:, bass.ts(h, 512)], start=True, stop=False)
                    nc.tensor.matmul(dh1[:, bass.ts(h, 512)], c_w2nb[:],
                                     da2b[:, bass.ts(h, 512)], start=False, stop=True)
                h1sq = sb.tile([128, b], BF16, tag="H1sq")
                nc.vector.tensor_mul(h1sq[:], h1[:], h1[:])
                da1 = sb.tile([128, b], BF16, tag="dA1")
                nc.vector.scalar_tensor_tensor(
                    da1[:], h1sq[:], 1.0, dh1[:], ALU.subtract, ALU.mult)
                dx = psA.tile([64, b], F32, tag="psA")
                for h in range(b // 512):
                    nc.tensor.matmul(dx[:, bass.ts(h, 512)], c_w1n[:],
                                     da1[:, bass.ts(h, 512)], start=True, stop=True)

                # ---- assemble + transpose back + store ----
                outt = sb.tile([128, b], F32, tag="OUTT")
                nc.scalar.copy(outt[0:64, :], dx[:, :])
                nc.scalar.copy(outt[64:128, :], dv[:, :])
                ot = psB.tile([128, b], F32, tag="psB")
                for k in range(g):
                    nc.tensor.transpose(ot[:, bass.ts(k, 128)],
                                        outt[:, bass.ts(k, 128)], c_id[:])
                outs = sb.tile([128, b], F32, tag="OUTS")
                nc.scalar.copy(outs[:], ot[:])
                nc.sync.dma_start(
                    outp[bass.ts(t, b), :].rearrange("(g p) f -> p g f", p=128),
                    outs[:].rearrange("p (g f) -> p g f", f=128),
                )

    nc.finalize()
    return nc


def make_consts(W1, b1, W2, b2):
    """Host-side constant preparation (permutes W2 rows, folds signs/scales)."""
    bf = ml_dtypes.bfloat16
    W1 = np.asarray(W1, np.float32)
    b1 = np.asarray(b1, np.float32)
    W2 = np.asarray(W2, np.float32)
    b2 = np.asarray(b2, np.float32)
    perm = np.empty(RANK * D, np.int64)
    for j in range(RANK):
        for i in range(D):
            perm[j * D + i] = i * RANK + j
    W2r = W2[perm, :]
    b2r = b2[perm]
    mblk = np.zeros((128, 128), np.float32)
    mblk[:64, :64] = 2.0
    mblk[64:, 64:] = 2.0
    msum = np.zeros((128, 64), np.float32)
    for i in range(64):
        msum[i, i] = -1.0
        msum[64 + i, i] = -1.0
    return {
        "w1t": np.ascontiguousarray(W1.T).astype(bf),
        "w2ta": np.ascontiguousarray(W2r[:128].T).astype(bf),
        "w2tb": np.ascontiguousarray(W2r[128:].T).astype(bf),
        "w2na": np.ascontiguousarray(-W2r[:128]).astype(bf),
        "w2nb": np.ascontiguousarray(-W2r[128:]).astype(bf),
        "w1n": np.ascontiguousarray(-W1).astype(bf),
        "mblk": mblk.astype(bf),
        "msum": msum.astype(bf),
        "ident": np.eye(128, dtype=np.float32),
        "b1": b1.reshape(128, 1).astype(np.float32),
        "b2a": b2r[:128].reshape(128, 1).astype(np.float32),
        "b2b": b2r[128:].reshape(128, 1).astype(np.float32),
    }


_NC_CACHE = {}


def _get_program(n_rows, b):
    key = (n_rows, b)
    if key not in _NC_CACHE:
        _NC_CACHE[key] = build_program(n_rows, b)
    return _NC_CACHE[key]


def kernel(t, input_, W1, b1, W2, b2):
    input_ = np.asarray(input_, np.float32)
    n = input_.shape[0]
    n_rows = n // N_CORES
    consts = make_consts(W1, b1, W2, b2)
    nc = _get_program(n_rows, B)
    in_maps = []
    for c in range(N_CORES):
        m = {"inp": np.ascontiguousarray(input_[c * n_rows:(c + 1) * n_rows])}
        m.update(consts)
        in_maps.append(m)
    res = run_bass_kernel_spmd(nc, in_maps, list(range(N_CORES)))
    out = np.concatenate([np.asarray(res.results[c]["out"]) for c in range(N_CORES)],
                         axis=0)
    return out.astype(np.float32)
